# revision 18
# baseline (speedup 1.0000x reference)
"""Trainium2 Bass kernel for nn_CapsuleLayerTSV (capsule routing over 40 adapters).

Strategy (8 NeuronCores, two SPMD NEFFs, no collectives):
  Phase 1 (expert-parallel): allowed adapters (tsv[t] != 0) sharded across
    cores. Priors computed in fp16 with 2-term error compensation
    (xh@Wh + xl@Wh; W's fp16 rounding alone contributes ~1e-4 relative to
    priors, amplified ~30x by the routing softmax to ~9e-3 final -- inside
    the 2e-2 gate). fp16 matmuls run 1 cyc/row vs fp32's 4. Each core also
    emits S = sum of its adapters' priors on the otherwise-idle GpSimd so
    the iteration-1 uniform vote never has to be summed in phase 2.
  Host: reassemble priors, re-shard by the OUTPUT's flat row space (the torch
    .view(bz, 200, 3) scramble means output row r uses flat vote elements
    3r..3r+2, so core c needs pairs s in [96c, 96c+96), s = n*256 + b).
  Phase 2 (pair-parallel): 3-iteration dynamic routing with the squash factor
    folded into the agreement ops as a per-partition scalar (o_i is never
    materialized), 21-term sweeps split Vector(14)/GpSimd(7), softmax's
    subtract fused into the ACT exp via bias=-rowmax, then a bf16 projection
    u[6400,3] @ (larger_w*g).T streamed out in 5-j-chunk batches at HBM line
    rate.
"""

import sys

sys.path.insert(0, "/opt/trn_rl_repo")

import numpy as np
import ml_dtypes

import concourse.bass as bass
import concourse.mybir as mybir
import concourse.tile as tile
from concourse.bass_utils import run_bass_kernel_spmd

F32 = mybir.dt.float32
BF16 = mybir.dt.bfloat16
FP16 = mybir.dt.float16
AX = mybir.AxisListType
ALU = mybir.AluOpType
ACTF = mybir.ActivationFunctionType

NC = 8
B = 256
ADP = 40
CAPS = 3
INCH = 600
D = 200
M = 768
ND = CAPS * D  # 600
PP = CAPS * B // NC  # 96 (n,b) pairs per core in phase 2
ROWS_PER_CORE = PP * D // CAPS  # 6400 output rows per core
JCH = ROWS_PER_CORE // 128  # 50 j-chunks

_K_CHUNKS = [(0, 128), (128, 128), (256, 128), (384, 128), (512, 88)]
XW_W = 2 * B + ND  # 1112: [xh^T | xl^T | Wh]

_BUILD_CACHE = {}


def _split_multiwait_waits(nc):
    """walrus caps sync-waits at ONE per instruction. For instructions executed
    by an in-order engine sequencer (everything except queue-executed DMAs),
    splitting the wait list across preceding 1-wait NoOps/Drains on the same
    engine is semantics-preserving."""
    for fn in nc.m.functions:
        for blk in fn.blocks:
            out = []
            for inst in blk.instructions:
                si = getattr(inst, "sync_info", None)
                if (
                    si is not None
                    and si.on_wait
                    and len(si.on_wait) > 1
                    and not isinstance(inst, mybir.InstDMACopy)
                    and getattr(inst, "engine", None) is not None
                ):
                    waits = list(si.on_wait)
                    cls = (
                        mybir.InstDrain
                        if isinstance(inst, mybir.InstDrain)
                        else mybir.InstNoOp
                    )
                    for i, w in enumerate(waits[:-1]):
                        extra = cls(
                            name=f"{inst.name}_w{i}",
                            engine=inst.engine,
                            sync_info=mybir.SyncInfo(on_wait=[w], on_update=[]),
                            bass_nofuse=True,
                        )
                        nc.register_instruction(extra)
                        out.append(extra)
                    si.on_wait = waits[-1:]
                out.append(inst)
            blk.instructions = out


# test/debug hook: kernel() appends the BassKernelResults of each phase here
LAST_RESULTS = []


def _build_phase1(ka):
    """SPMD program: fp16 2-term priors for `ka` adapter slots per core.

    inputs : xw  [ka, 600, 1112] fp16  (cols 0:256 xh^T, 256:512 xl^T,
                                        512:1112 Wh [c, n*d])
    outputs: pri [ka, 2, 128, 600] f32  (priors, b in 2 chunks of 128)
             S   [2, 128, 600] f32      (sum of this core's adapters' priors)
    """
    nc = bass.Bass()
    xw = nc.declare_dram_parameter("xw", [ka, INCH, XW_W], FP16, isOutput=False)
    pri = nc.declare_dram_parameter("pri", [ka, 2, 128, ND], F32, isOutput=True)
    S = nc.declare_dram_parameter("S", [2, 128, ND], F32, isOutput=True)

    with tile.TileContext(nc) as tc:
        with (
            tc.tile_pool(name="xt", bufs=1) as xt_pool,
            tc.tile_pool(name="ob", bufs=2 * ka) as ob_pool,
            tc.tile_pool(name="ssb", bufs=1) as s_pool,
            tc.tile_pool(name="ps", bufs=2, space="PSUM") as ps_pool,
        ):
            # Wait-budget discipline (walrus: max ONE sync-wait per
            # instruction): junk [1,1] matmuls absorb each chunk-DMA tick into
            # PE's observed clock, so real matmuls only ever wait on the PSUM
            # slot release. Output tiles are never reused (bufs=2*ka).
            ps_junk = ps_pool.tile([1, 1], F32, tag="pjunk", bufs=1)
            S_sb = [
                s_pool.tile([128, ND], F32, tag=f"S{bc}", name=f"S{bc}")
                for bc in range(2)
            ]
            for k in range(ka):
                xwts = []
                for ci, (c0, cs) in enumerate(_K_CHUNKS):
                    xw_t = xt_pool.tile(
                        [cs, XW_W], FP16, tag=f"xw{k}_{ci}", name=f"xw{k}_{ci}"
                    )
                    nc.sync.dma_start(out=xw_t[:, :], in_=xw[k, c0 : c0 + cs, :])
                    nc.tensor.matmul(
                        ps_junk[:, :], xw_t[0:1, 0:1], xw_t[0:1, 0:1],
                        start=True, stop=True,
                    )
                    xwts.append(xw_t)
                for bc in range(2):
                    pss = [
                        ps_pool.tile([128, ND // 2], F32, tag=f"ps{gi}", name=f"ps{gi}")
                        for gi in range(2)
                    ]
                    for ci in range(len(_K_CHUNKS)):
                        for ti in range(2):  # hi, lo term of x
                            xoff = ti * B + bc * 128
                            for gi in range(2):
                                nc.tensor.matmul(
                                    pss[gi][:, :],
                                    xwts[ci][:, xoff : xoff + 128],
                                    xwts[ci][
                                        :, 2 * B + gi * 300 : 2 * B + (gi + 1) * 300
                                    ],
                                    start=(ci == 0 and ti == 0),
                                    stop=(ci == len(_K_CHUNKS) - 1 and ti == 1),
                                )
                    osb = ob_pool.tile([128, ND], F32, tag="osb")
                    nc.vector.tensor_copy(osb[:, :300], pss[0][:, :])
                    nc.vector.tensor_copy(osb[:, 300:], pss[1][:, :])
                    # iter-1 vote partial sum (DVE: GpSimd's 2-input ops are
                    # ~3x slower and trailed the matmuls by ~12us)
                    if k == 0:
                        nc.vector.tensor_copy(S_sb[bc][:, :], osb[:, :])
                    else:
                        nc.vector.tensor_tensor(
                            out=S_sb[bc][:, :], in0=S_sb[bc][:, :], in1=osb[:, :],
                            op=ALU.add,
                        )
                    # GP absorber pulls the DVE tick so the store has <=1 wait
                    pabs = ob_pool.tile(
                        [1, 1], F32, tag=f"pa{k}_{bc}", name=f"pa{k}_{bc}", bufs=1
                    )
                    nc.gpsimd.tensor_copy(pabs[:, :], osb[0:1, ND - 1 : ND])
                    nc.gpsimd.dma_start(out=pri[k, bc, :, :], in_=osb[:, :])
            for bc in range(2):
                sabs = ob_pool.tile(
                    [1, 1], F32, tag=f"sa{bc}", name=f"sa{bc}", bufs=1
                )
                nc.gpsimd.tensor_copy(sabs[:, :], S_sb[bc][0:1, ND - 1 : ND])
                nc.gpsimd.dma_start(out=S[bc, :, :], in_=S_sb[bc][:, :])
    return nc


def _build_phase2(A):
    """SPMD program: routing for 96 (n,b) pairs + output projection per core.

    inputs : pri2 [96, A*200] f32  (priors for this core's pairs, k-major)
             vs1i [96, 200] f32    (sum over allowed k of priors = A*vote_1)
             lwg  [3, 768] bf16    (gated projection matrix, plain bf16)
    output : outc [6400, 768] f32

    Assumes tsv[t, allowed] == 1 (checked on host), so the reference's
    logits*tsv multiplies are identity for every adapter we process.
    """
    nc = bass.Bass()
    pri2 = nc.declare_dram_parameter("pri2", [PP, A * D], FP16, isOutput=False)
    vs1i = nc.declare_dram_parameter("vs1i", [PP, D], F32, isOutput=False)
    lwg = nc.declare_dram_parameter("lwg", [CAPS, M], BF16, isOutput=False)
    outc = nc.declare_dram_parameter("outc", [ROWS_PER_CORE, M], F32, isOutput=True)

    inv_a = 1.0 / float(A)
    inv_a2 = inv_a * inv_a
    uid = [0]
    DVK = 14  # adapters 0:DVK on Vector, DVK:A on GpSimd per sweep

    with tile.TileContext(nc) as tc:
        with (
            tc.tile_pool(name="ps", bufs=2, space="PSUM") as ps_pool,
            tc.tile_pool(name="ob", bufs=2) as ob_pool,
        ):
            _sb_cm = tc.tile_pool(name="sb", bufs=1)
            sb = _sb_cm.__enter__()

            def fresh(shape, dtype=F32, pfx="t", pool=None):
                uid[0] += 1
                p = pool if pool is not None else sb
                return p.tile(shape, dtype, tag=f"{pfx}{uid[0]}", name=f"{pfx}{uid[0]}")

            def absorb_dve(ap):
                s = fresh([1, 1], ap.dtype, "slv")
                nc.vector.tensor_copy(s[:, :], ap[0:1, 0:1])

            def absorb_act(ap):
                s = fresh([1, 1], ap.dtype, "sla")
                nc.scalar.copy(s[:, :], ap[0:1, 0:1])

            def absorb_gp(ap):
                s = fresh([1, 1], ap.dtype, "slg")
                nc.gpsimd.tensor_copy(s[:, :], ap[0:1, 0:1])

            # ---- ACT table warmup: exp+sqrt tables load during the P DMA ----
            warm = fresh([1, 1], F32, "wrm")
            warm2 = fresh([1, 1], F32, "wr2")
            nc.vector.memset(warm[:, :], 1.0)
            absorb_act(warm)
            nc.scalar.activation(warm2[:, :], warm[:, :], ACTF.Exp)
            nc.scalar.sqrt(warm2[:, :], warm[:, :])

            # ---- loads: P (fp16) split across both HWDGE rings, first ----
            P = sb.tile([PP, A * D], FP16, tag="P")
            Pv = P[:, :].rearrange("p (k d) -> p k d", k=A)
            HK = A // 2
            nc.sync.dma_start(out=P[:, : HK * D], in_=pri2[:, : HK * D])
            nc.scalar.dma_start(out=P[:, HK * D :], in_=pri2[:, HK * D :])
            vs1 = sb.tile([PP, D], F32, tag="vs1")
            nc.sync.dma_start(out=vs1[:, :], in_=vs1i[:, :])
            absorb_dve(vs1)
            lwg_t = sb.tile([CAPS, M], BF16, tag="lwg")
            nc.scalar.dma_start(out=lwg_t[:, :], in_=lwg[:, :])
            absorb_dve(P)     # pull both P-DMA ticks into DVE's clock
            pabP = fresh([1, 1], FP16, "pgp")
            nc.gpsimd.tensor_copy(pabP[:, :], P[0:1, A * D - 1 : A * D])

            def squash_factor(v_t, pre_scale_sq, post_scale):
                """f = post_scale * sqrt(t)/(1+t), t = sum(v_t^2)*pre_scale_sq.
                pre_scale_sq/post_scale: float or [PP,1] AP."""
                junk = fresh([PP, D], F32, "sqj")
                sq = fresh([PP, 1], F32, "sq")
                t = fresh([PP, 1], F32, "tt")
                s = fresh([PP, 1], F32, "ss")
                u = fresh([PP, 1], F32, "uu")
                r = fresh([PP, 1], F32, "rr")
                f = fresh([PP, 1], F32, "ff")
                nc.vector.scalar_tensor_tensor(
                    out=junk[:, :], in0=v_t[:, :], scalar=1.0, in1=v_t[:, :],
                    op0=ALU.mult, op1=ALU.mult, accum_out=sq[:, 0:1],
                )
                if isinstance(pre_scale_sq, float):
                    nc.vector.tensor_scalar(
                        out=t[:, :], in0=sq[:, :], scalar1=pre_scale_sq,
                        scalar2=None, op0=ALU.mult,
                    )
                else:
                    nc.vector.tensor_tensor(
                        out=t[:, :], in0=sq[:, :], in1=pre_scale_sq, op=ALU.mult
                    )
                nc.scalar.sqrt(s[:, :], t[:, :])
                nc.vector.tensor_scalar(
                    out=u[:, :], in0=t[:, :], scalar1=1.0, scalar2=None, op0=ALU.add,
                )
                nc.vector.reciprocal(r[:, :], u[:, :])
                absorb_dve(s)  # pull the ACT sqrt tick before the fused f op
                nc.vector.scalar_tensor_tensor(
                    out=f[:, :], in0=s[:, :], scalar=post_scale, in1=r[:, :],
                    op0=ALU.mult, op1=ALU.mult,
                )
                return f

            def agreement(v_t, aT):
                """aT[:, k] = raw sum_d P[:,k,:] * v_t (DVE fused mult+accum;
                Pool supports neither accum_out nor free-axis reduce)."""
                junkv = fresh([PP, D], F32, "agv")
                for k in range(A):
                    nc.vector.scalar_tensor_tensor(
                        out=junkv[:, :], in0=Pv[:, k, :], scalar=1.0,
                        in1=v_t[:, :], op0=ALU.mult, op1=ALU.mult,
                        accum_out=aT[:, k : k + 1],
                    )

            def softmax(logit):
                """returns (e, dinv): e = exp(logit - max), dinv = 1/sum(e)."""
                rmax = fresh([PP, 1], F32, "rmx")
                nmx = fresh([PP, 1], F32, "nmx")
                e = fresh([PP, A], F32, "e")
                dsum = fresh([PP, 1], F32, "dsm")
                dinv = fresh([PP, 1], F32, "dnv")
                nc.vector.tensor_reduce(rmax[:, :], logit[:, :], AX.X, ALU.max)
                nc.vector.tensor_scalar(
                    out=nmx[:, :], in0=rmax[:, :], scalar1=-1.0, scalar2=None,
                    op0=ALU.mult,
                )
                absorb_act(nmx)  # ACT waits once on DVE tick, then exp is clean
                nc.scalar.activation(
                    e[:, :], logit[:, :], ACTF.Exp, bias=nmx[:, 0:1],
                    accum_out=dsum[:, 0:1],
                )
                absorb_dve(e)
                nc.vector.reciprocal(dinv[:, :], dsum[:, :])
                return e, dinv

            def vote(e):
                """acc = sum_k e[:,k] * P[:,k,:]: two interleaved DVE chains."""
                acca = fresh([PP, D], F32, "vca")
                accb = fresh([PP, D], F32, "vcb")
                vs = fresh([PP, D], F32, "vss")
                nc.vector.tensor_scalar(
                    out=acca[:, :], in0=Pv[:, 0, :], scalar1=e[:, 0:1],
                    scalar2=None, op0=ALU.mult,
                )
                nc.vector.tensor_scalar(
                    out=accb[:, :], in0=Pv[:, 1, :], scalar1=e[:, 1:2],
                    scalar2=None, op0=ALU.mult,
                )
                for k in range(2, A):
                    acc = acca if k % 2 == 0 else accb
                    nc.vector.scalar_tensor_tensor(
                        out=acc[:, :], in0=Pv[:, k, :], scalar=e[:, k : k + 1],
                        in1=acc[:, :], op0=ALU.mult, op1=ALU.add,
                    )
                nc.vector.tensor_tensor(
                    out=vs[:, :], in0=acca[:, :], in1=accb[:, :], op=ALU.add
                )
                return vs

            # ---- iteration 1: probs uniform over allowed; vote1 = vs1/A ----
            # agreement with raw v (squash factor f folded at the logit step:
            # <P_k, o> = f * <P_k, v>)
            f1 = squash_factor(vs1, inv_a2, inv_a)
            aT1 = fresh([PP, A], F32, "aT1")
            agreement(vs1, aT1)
            logit1 = fresh([PP, A], F32, "lg1")
            absorb_dve(aT1)  # GP wrote aT1 tail columns
            nc.vector.tensor_scalar(
                out=logit1[:, :], in0=aT1[:, :], scalar1=f1[:, 0:1],
                scalar2=None, op0=ALU.mult,
            )  # tsv multiply is identity for allowed adapters

            # ---- iteration 2 ----
            e2, dinv2 = softmax(logit1)
            vs2 = vote(e2)
            d2 = fresh([PP, 1], F32, "d2")
            nc.vector.tensor_tensor(
                out=d2[:, :], in0=dinv2[:, :], in1=dinv2[:, :], op=ALU.mult
            )
            f2 = squash_factor(vs2, d2[:, 0:1], dinv2[:, 0:1])
            aT2 = fresh([PP, A], F32, "aT2")
            agreement(vs2, aT2)
            logit2 = fresh([PP, A], F32, "lg2")
            absorb_dve(aT2)  # GP wrote aT2 tail columns
            nc.vector.scalar_tensor_tensor(
                out=logit2[:, :], in0=aT2[:, :], scalar=f2[:, 0:1],
                in1=logit1[:, :], op0=ALU.mult, op1=ALU.add,
            )

            # ---- iteration 3: final vote, scaled by 1/sum(e) ----
            e3, dinv3 = softmax(logit2)
            vs3 = vote(e3)
            v3h = fresh([PP, D], BF16, "v3h")
            nc.vector.tensor_scalar(
                out=v3h[:, :], in0=vs3[:, :], scalar1=dinv3[:, 0:1],
                scalar2=None, op0=ALU.mult,
            )

            # ---- deinterleave the flat vote stream into u^T rows ----
            # vote [96,200] -> [32,600] (3 pairs per partition = 600 flat
            # values) -> stride-3 in-partition deinterleave -> [3, 6400].
            vstack = fresh([PP // CAPS, CAPS * D], BF16, "vstk")
            nc.gpsimd.dma_start(
                out=vstack[:, :].rearrange("q (m d) -> q m d", m=CAPS),
                in_=v3h[:, :],
            )
            uT2 = fresh([PP // CAPS, CAPS * D], BF16, "uT2")
            nc.vector.tensor_copy(
                uT2[:, :].rearrange("q (k jl) -> q k jl", k=CAPS),
                vstack[:, :].rearrange("q (jl k) -> q k jl", k=CAPS),
            )
            uT = sb.tile([CAPS, ROWS_PER_CORE], BF16, tag="uT")
            for kk in range(CAPS):
                nc.gpsimd.dma_start(
                    out=uT[kk : kk + 1, :].rearrange(
                        "k (q jl) -> k q jl", q=PP // CAPS
                    ),
                    in_=uT2[:, kk * D : (kk + 1) * D],
                )

            # PE absorbers: junk matmuls ladder the uT-writer + lwg ticks into
            # PE's clock (dep tracking is byte-range based)
            ps_junk = ps_pool.tile([1, 1], F32, tag="pjunk", bufs=1)
            for labs in (lwg_t[0:1, 0:1], uT[0:1, 0:1]):
                nc.tensor.matmul(ps_junk[:, :], labs, labs, start=True, stop=True)

            # ---- projection: out[j, :] = uT[:, j].T @ lwg ----
            # First batches are small so the store stream starts early.
            HM = M // 2
            BATCHES = [2, 3] + [5] * ((JCH - 5) // 5)
            assert sum(BATCHES) == JCH
            last_pab = None
            jc = 0
            for bt, bch in enumerate(BATCHES):
                if last_pab is not None:
                    absorb_dve(last_pab)
                    absorb_act(last_pab)
                osb = ob_pool.tile([128, 5 * M], F32, tag="osb", name="osb")
                for ji in range(bch):
                    js = jc * 128
                    co = ji * M
                    psA = ps_pool.tile([128, HM], F32, tag="psA", name="psA")
                    psB = ps_pool.tile([128, HM], F32, tag="psB", name="psB")
                    nc.tensor.matmul(
                        psA[:, :], uT[:, js : js + 128], lwg_t[:, :HM],
                        start=True, stop=True,
                    )
                    nc.tensor.matmul(
                        psB[:, :], uT[:, js : js + 128], lwg_t[:, HM:],
                        start=True, stop=True,
                    )
                    if ji == 0:
                        absorb_dve(psA)
                        absorb_act(psB)
                    nc.vector.tensor_copy(osb[:, co : co + HM], psA[:, :])
                    nc.scalar.copy(osb[:, co + HM : co + M], psB[:, :])
                    jc += 1
                r0 = (jc - bch) * 128
                src = osb[:, : bch * M].rearrange("p (j m) -> p j m", j=bch)
                dst = outc[r0 : r0 + bch * 128, :].rearrange("(j p) m -> p j m", p=128)
                pab = fresh([1, 2 * bch], F32, "pba")
                nc.gpsimd.tensor_copy(pab[:, :], osb[0:1, 0 : bch * M : HM])
                nc.gpsimd.dma_start(out=dst, in_=src)
                last_pab = pab
            _sb_cm.__exit__(None, None, None)
    return nc


def _get_programs(A, ka):
    key = (A, ka)
    if key not in _BUILD_CACHE:
        nc1, nc2 = _build_phase1(ka), _build_phase2(A)
        _split_multiwait_waits(nc1)
        _split_multiwait_waits(nc2)
        _BUILD_CACHE[key] = (nc1, nc2)
    return _BUILD_CACHE[key]


def kernel(t, x, s, route_weights, larger_w, larger_b, elarger, tsv):
    t = int(t)
    x = np.ascontiguousarray(np.asarray(x, np.float32))
    tsv_t = np.asarray(tsv, np.float32)[t]
    allowed = np.nonzero(tsv_t != 0)[0]
    assert np.all(tsv_t[allowed] == 1.0), "non-binary tsv not supported"
    A = len(allowed)
    ka = (A + NC - 1) // NC

    nc1, nc2 = _get_programs(A, ka)

    # ---------- phase 1: priors, expert-parallel ----------
    rw = np.asarray(route_weights, np.float32)
    in1 = []
    for c in range(NC):
        xw_c = np.zeros((ka, INCH, XW_W), np.float16)
        for j in range(ka):
            g = c * ka + j
            if g < A:
                k = allowed[g]
                xT = x[:, k, :].T  # [600, 256]
                xh = xT.astype(np.float16)
                xw_c[j, :, :B] = xh
                xw_c[j, :, B : 2 * B] = (xT - xh.astype(np.float32)).astype(
                    np.float16
                )
                xw_c[j, :, 2 * B :] = (
                    rw[k].transpose(1, 0, 2).reshape(INCH, ND).astype(np.float16)
                )
        in1.append({"xw": xw_c})
    res1 = run_bass_kernel_spmd(nc1, in1, list(range(NC)))
    LAST_RESULTS.append(res1)

    # priors_full[k, b, n, d]; vote-1 numerator summed on device
    priors_full = np.zeros((A, B, CAPS, D), np.float32)
    vsum = np.zeros((2, 128, ND), np.float32)
    for c in range(NC):
        pri = res1.results[c]["pri"]  # [ka, 2, 128, 600]
        for j in range(ka):
            g = c * ka + j
            if g < A:
                priors_full[g] = pri[j].reshape(B, CAPS, D)
        vsum += res1.results[c]["S"]
    vsum_bnd = vsum.reshape(B, CAPS, D)

    # ---------- phase 2: routing + projection, pair-parallel ----------
    g_gate = 1.0 / (
        1.0 + np.exp(-(np.float32(s[0]) * np.asarray(elarger, np.float32)[t]))
    )
    lwg_f = np.asarray(larger_w, np.float32) * g_gate[:, None]  # [768, 3]
    bg = np.asarray(larger_b, np.float32) * g_gate  # [768]
    assert not np.any(bg), "nonzero larger_b not supported by this build"
    lwg_bf = lwg_f.T.astype(ml_dtypes.bfloat16)  # [3, 768]

    in2 = []
    for c in range(NC):
        sidx = np.arange(c * PP, (c + 1) * PP)
        nv, bv = sidx // B, sidx % B
        P2 = priors_full[:, bv, nv, :].transpose(1, 0, 2)  # [96, A, 200]
        in2.append(
            {
                "pri2": np.ascontiguousarray(
                    P2.reshape(PP, A * D).astype(np.float16)
                ),
                "vs1i": np.ascontiguousarray(vsum_bnd[bv, nv, :]),
                "lwg": lwg_bf,
            }
        )
    res2 = run_bass_kernel_spmd(nc2, in2, list(range(NC)))
    LAST_RESULTS.append(res2)

    out = np.concatenate([res2.results[c]["outc"] for c in range(NC)], axis=0)
    return out.reshape(B, D, M)


# revision 27
# speedup vs baseline: 1.0552x; 1.0552x over previous
"""Trainium2 Bass kernel for nn_CapsuleLayerTSV (capsule routing over 40 adapters).

Strategy (8 NeuronCores, two SPMD NEFFs, no collectives):
  Phase 1 (expert-parallel): allowed adapters (tsv[t] != 0) sharded across
    cores. Priors computed in fp16 with 2-term error compensation
    (xh@Wh + xl@Wh; W's fp16 rounding alone contributes ~1e-4 relative to
    priors, amplified ~30x by the routing softmax to ~9e-3 final -- inside
    the 2e-2 gate). fp16 matmuls run 1 cyc/row vs fp32's 4. Each core also
    emits S = sum of its adapters' priors on the otherwise-idle GpSimd so
    the iteration-1 uniform vote never has to be summed in phase 2.
  Host: reassemble priors, re-shard by the OUTPUT's flat row space (the torch
    .view(bz, 200, 3) scramble means output row r uses flat vote elements
    3r..3r+2, so core c needs pairs s in [96c, 96c+96), s = n*256 + b).
  Phase 2 (pair-parallel): 3-iteration dynamic routing with the squash factor
    folded into the agreement ops as a per-partition scalar (o_i is never
    materialized), 21-term sweeps split Vector(14)/GpSimd(7), softmax's
    subtract fused into the ACT exp via bias=-rowmax, then a bf16 projection
    u[6400,3] @ (larger_w*g).T streamed out in 5-j-chunk batches at HBM line
    rate.
"""

import sys

sys.path.insert(0, "/opt/trn_rl_repo")

import numpy as np
import ml_dtypes

import concourse.bass as bass
import concourse.mybir as mybir
import concourse.tile as tile
from concourse.bass_utils import run_bass_kernel_spmd

F32 = mybir.dt.float32
BF16 = mybir.dt.bfloat16
FP16 = mybir.dt.float16
AX = mybir.AxisListType
ALU = mybir.AluOpType
ACTF = mybir.ActivationFunctionType

NC = 8
B = 256
ADP = 40
CAPS = 3
INCH = 600
D = 200
M = 768
ND = CAPS * D  # 600
PP = CAPS * B // NC  # 96 (n,b) pairs per core in phase 2
ROWS_PER_CORE = PP * D // CAPS  # 6400 output rows per core
JCH = ROWS_PER_CORE // 128  # 50 j-chunks

_K_CHUNKS = [(0, 128), (128, 128), (256, 128), (384, 128), (512, 88)]
NKC = len(_K_CHUNKS)
XW_W = 2 * B + ND  # 1112: [xh^T | xl^T | Wh] per k-chunk

_BUILD_CACHE = {}


def _split_multiwait_waits(nc):
    """walrus caps sync-waits at ONE per instruction. For instructions executed
    by an in-order engine sequencer (everything except queue-executed DMAs),
    splitting the wait list across preceding 1-wait NoOps/Drains on the same
    engine is semantics-preserving."""
    for fn in nc.m.functions:
        for blk in fn.blocks:
            out = []
            for inst in blk.instructions:
                si = getattr(inst, "sync_info", None)
                if (
                    si is not None
                    and si.on_wait
                    and len(si.on_wait) > 1
                    and not isinstance(inst, mybir.InstDMACopy)
                    and getattr(inst, "engine", None) is not None
                ):
                    waits = list(si.on_wait)
                    cls = (
                        mybir.InstDrain
                        if isinstance(inst, mybir.InstDrain)
                        else mybir.InstNoOp
                    )
                    for i, w in enumerate(waits[:-1]):
                        extra = cls(
                            name=f"{inst.name}_w{i}",
                            engine=inst.engine,
                            sync_info=mybir.SyncInfo(on_wait=[w], on_update=[]),
                            bass_nofuse=True,
                        )
                        nc.register_instruction(extra)
                        out.append(extra)
                    si.on_wait = waits[-1:]
                out.append(inst)
            blk.instructions = out


# test/debug hook: kernel() appends the BassKernelResults of each phase here
LAST_RESULTS = []


def _build_phase1(ka):
    """SPMD program: fp16 2-term priors for `ka` adapter slots per core.

    inputs : xw  [ka, 128, 5*1112] fp16 -- per k-chunk ci the column block
             [ci*1112, (ci+1)*1112) holds [xh^T | xl^T | Wh], rows past the
             chunk's K zero-padded (zero rows accumulate nothing)
    outputs: pri [ka, 2, 128, 600] f32  (priors, b in 2 chunks of 128)
             S   [2, 128, 600] f32      (sum of this core's adapters' priors)
    """
    nc = bass.Bass()
    xw = nc.declare_dram_parameter("xw", [ka, 128, NKC * XW_W], FP16, isOutput=False)
    pri = nc.declare_dram_parameter("pri", [ka, 2, 128, ND], F32, isOutput=True)
    S = nc.declare_dram_parameter("S", [2, 128, ND], F32, isOutput=True)

    with tile.TileContext(nc) as tc:
        with (
            tc.tile_pool(name="xt", bufs=1) as xt_pool,
            tc.tile_pool(name="ob", bufs=2 * ka) as ob_pool,
            tc.tile_pool(name="ssb", bufs=1) as s_pool,
            tc.tile_pool(name="ps", bufs=2, space="PSUM") as ps_pool,
        ):
            # Wait-budget discipline (walrus: max ONE sync-wait per
            # instruction): junk [1,1] matmuls absorb each DMA tick into
            # PE's observed clock, so real matmuls only ever wait on the PSUM
            # slot release. Output tiles are never reused (bufs=2*ka).
            ps_junk = ps_pool.tile([1, 1], F32, tag="pjunk", bufs=1)
            S_sb = [
                s_pool.tile([128, ND], F32, tag=f"S{bc}", name=f"S{bc}")
                for bc in range(2)
            ]
            for k in range(ka):
                xw_t = xt_pool.tile(
                    [128, NKC * XW_W], FP16, tag=f"xw{k}", name=f"xw{k}"
                )
                nc.sync.dma_start(out=xw_t[:, :], in_=xw[k, :, :])
                nc.tensor.matmul(
                    ps_junk[:, :], xw_t[0:1, 0:1], xw_t[0:1, 0:1],
                    start=True, stop=True,
                )
                for bc in range(2):
                    pss = [
                        ps_pool.tile([128, ND // 2], F32, tag=f"ps{gi}", name=f"ps{gi}")
                        for gi in range(2)
                    ]
                    for ci in range(NKC):
                        c0 = ci * XW_W
                        for ti in range(2):  # hi, lo term of x
                            xoff = c0 + ti * B + bc * 128
                            for gi in range(2):
                                nc.tensor.matmul(
                                    pss[gi][:, :],
                                    xw_t[:, xoff : xoff + 128],
                                    xw_t[
                                        :,
                                        c0 + 2 * B + gi * 300 : c0
                                        + 2 * B
                                        + (gi + 1) * 300,
                                    ],
                                    start=(ci == 0 and ti == 0),
                                    stop=(ci == NKC - 1 and ti == 1),
                                )
                    osb = ob_pool.tile([128, ND], F32, tag="osb")
                    nc.vector.tensor_copy(osb[:, :300], pss[0][:, :])
                    nc.vector.tensor_copy(osb[:, 300:], pss[1][:, :])
                    # iter-1 vote partial sum (DVE: GpSimd's 2-input ops are
                    # ~3x slower and trailed the matmuls by ~12us)
                    if k == 0:
                        nc.vector.tensor_copy(S_sb[bc][:, :], osb[:, :])
                    else:
                        nc.vector.tensor_tensor(
                            out=S_sb[bc][:, :], in0=S_sb[bc][:, :], in1=osb[:, :],
                            op=ALU.add,
                        )
                    # GP absorber pulls the DVE tick so the store has <=1 wait
                    pabs = ob_pool.tile(
                        [1, 1], F32, tag=f"pa{k}_{bc}", name=f"pa{k}_{bc}", bufs=1
                    )
                    nc.gpsimd.tensor_copy(pabs[:, :], osb[0:1, ND - 1 : ND])
                    nc.gpsimd.dma_start(out=pri[k, bc, :, :], in_=osb[:, :])
            for bc in range(2):
                sabs = ob_pool.tile(
                    [1, 1], F32, tag=f"sa{bc}", name=f"sa{bc}", bufs=1
                )
                nc.gpsimd.tensor_copy(sabs[:, :], S_sb[bc][0:1, ND - 1 : ND])
                nc.gpsimd.dma_start(out=S[bc, :, :], in_=S_sb[bc][:, :])
    return nc


def _build_phase2(A):
    """SPMD program: routing for 96 (n,b) pairs + output projection per core.

    inputs : pri2 [96, A*200] f32  (priors for this core's pairs, k-major)
             vs1i [96, 200] f32    (sum over allowed k of priors = A*vote_1)
             lwg  [3, 768] bf16    (gated projection matrix, plain bf16)
    output : outc [6400, 768] f32

    Assumes tsv[t, allowed] == 1 (checked on host), so the reference's
    logits*tsv multiplies are identity for every adapter we process.
    """
    nc = bass.Bass()
    pri2 = nc.declare_dram_parameter("pri2", [PP, A * D], FP16, isOutput=False)
    vs1i = nc.declare_dram_parameter("vs1i", [PP, D], F32, isOutput=False)
    lwg = nc.declare_dram_parameter("lwg", [CAPS, M], BF16, isOutput=False)
    outc = nc.declare_dram_parameter("outc", [ROWS_PER_CORE, M], F32, isOutput=True)

    inv_a = 1.0 / float(A)
    inv_a2 = inv_a * inv_a
    uid = [0]
    DVK = 14  # adapters 0:DVK on Vector, DVK:A on GpSimd per sweep

    with tile.TileContext(nc) as tc:
        with (
            tc.tile_pool(name="ps", bufs=2, space="PSUM") as ps_pool,
            tc.tile_pool(name="ob", bufs=3) as ob_pool,
        ):
            _sb_cm = tc.tile_pool(name="sb", bufs=1)
            sb = _sb_cm.__enter__()

            def fresh(shape, dtype=F32, pfx="t", pool=None):
                uid[0] += 1
                p = pool if pool is not None else sb
                return p.tile(shape, dtype, tag=f"{pfx}{uid[0]}", name=f"{pfx}{uid[0]}")

            def absorb_dve(ap):
                s = fresh([1, 1], ap.dtype, "slv")
                nc.vector.tensor_copy(s[:, :], ap[0:1, 0:1])

            def absorb_act(ap):
                s = fresh([1, 1], ap.dtype, "sla")
                nc.scalar.copy(s[:, :], ap[0:1, 0:1])

            def absorb_gp(ap):
                s = fresh([1, 1], ap.dtype, "slg")
                nc.gpsimd.tensor_copy(s[:, :], ap[0:1, 0:1])

            # ---- ACT table warmup: exp+sqrt tables load during the P DMA ----
            warm = fresh([1, 1], F32, "wrm")
            warm2 = fresh([1, 1], F32, "wr2")
            nc.vector.memset(warm[:, :], 1.0)
            absorb_act(warm)
            nc.scalar.activation(warm2[:, :], warm[:, :], ACTF.Exp)
            nc.scalar.sqrt(warm2[:, :], warm[:, :])

            # ---- loads: vs1 first (tiny, gates f1), then P across both
            # HWDGE rings in 4 slices ----
            vs1 = sb.tile([PP, D], F32, tag="vs1")
            nc.sync.dma_start(out=vs1[:, :], in_=vs1i[:, :])
            P = sb.tile([PP, A * D], FP16, tag="P")
            Pv = P[:, :].rearrange("p (k d) -> p k d", k=A)
            QK = [(0, 5), (5, 10), (10, 16), (16, A)]
            for qi, (k0, k1) in enumerate(QK):
                eng = nc.sync if qi % 2 == 0 else nc.scalar
                eng.dma_start(
                    out=P[:, k0 * D : k1 * D], in_=pri2[:, k0 * D : k1 * D]
                )
            absorb_dve(vs1)
            lwg_t = sb.tile([CAPS, M], BF16, tag="lwg")
            nc.scalar.dma_start(out=lwg_t[:, :], in_=lwg[:, :])
            for _ in range(3):
                absorb_dve(P)  # pull the four P-DMA ticks into DVE's clock
            pabP = fresh([1, 1], FP16, "pgp")
            nc.gpsimd.tensor_copy(pabP[:, :], P[0:1, A * D - 1 : A * D])
            vs1h = fresh([PP, D], FP16, "v1h")
            nc.vector.tensor_copy(vs1h[:, :], vs1[:, :])

            def squash_factor(v_t, pre_scale_sq, post_scale):
                """f = post_scale * sqrt(t)/(1+t), t = sum(v_t^2)*pre_scale_sq.
                pre_scale_sq/post_scale: float or [PP,1] AP."""
                junk = fresh([PP, D], F32, "sqj")
                sq = fresh([PP, 1], F32, "sq")
                t = fresh([PP, 1], F32, "tt")
                s = fresh([PP, 1], F32, "ss")
                u = fresh([PP, 1], F32, "uu")
                r = fresh([PP, 1], F32, "rr")
                f = fresh([PP, 1], F32, "ff")
                nc.vector.scalar_tensor_tensor(
                    out=junk[:, :], in0=v_t[:, :], scalar=1.0, in1=v_t[:, :],
                    op0=ALU.mult, op1=ALU.mult, accum_out=sq[:, 0:1],
                )
                if isinstance(pre_scale_sq, float):
                    nc.vector.tensor_scalar(
                        out=t[:, :], in0=sq[:, :], scalar1=pre_scale_sq,
                        scalar2=None, op0=ALU.mult,
                    )
                else:
                    nc.vector.tensor_tensor(
                        out=t[:, :], in0=sq[:, :], in1=pre_scale_sq, op=ALU.mult
                    )
                nc.scalar.sqrt(s[:, :], t[:, :])
                nc.vector.tensor_scalar(
                    out=u[:, :], in0=t[:, :], scalar1=1.0, scalar2=None, op0=ALU.add,
                )
                nc.vector.reciprocal(r[:, :], u[:, :])
                absorb_dve(s)  # pull the ACT sqrt tick before the fused f op
                nc.vector.scalar_tensor_tensor(
                    out=f[:, :], in0=s[:, :], scalar=post_scale, in1=r[:, :],
                    op0=ALU.mult, op1=ALU.mult,
                )
                return f

            def agreement(v16, aT):
                """aT[:, k] = raw sum_d P[:,k,:] * v16: ONE broadcast multiply
                (fp16 everywhere -> DVE 2x mode) + ONE segmented X-reduce per
                half-k slice (halves bound the op latency)."""
                AH = A // 2
                for k0, k1 in ((0, AH), (AH, A)):
                    kk = k1 - k0
                    prod = fresh([PP, kk * D], FP16, "agp")
                    pv3 = prod[:, :].rearrange("p (k d) -> p k d", k=kk)
                    nc.vector.tensor_tensor(
                        out=pv3,
                        in0=Pv[:, k0:k1, :],
                        in1=v16[:, :].unsqueeze(1).broadcast_to([PP, kk, D]),
                        op=ALU.mult,
                    )
                    nc.vector.tensor_reduce(aT[:, k0:k1], pv3, AX.X, ALU.add)

            def softmax(logit):
                """returns (e, dinv): e = exp(logit - max), dinv = 1/sum(e)."""
                rmax = fresh([PP, 1], F32, "rmx")
                nmx = fresh([PP, 1], F32, "nmx")
                e = fresh([PP, A], F32, "e")
                dsum = fresh([PP, 1], F32, "dsm")
                dinv = fresh([PP, 1], F32, "dnv")
                nc.vector.tensor_reduce(rmax[:, :], logit[:, :], AX.X, ALU.max)
                nc.vector.tensor_scalar(
                    out=nmx[:, :], in0=rmax[:, :], scalar1=-1.0, scalar2=None,
                    op0=ALU.mult,
                )
                absorb_act(nmx)  # ACT waits once on DVE tick, then exp is clean
                nc.scalar.activation(
                    e[:, :], logit[:, :], ACTF.Exp, bias=nmx[:, 0:1],
                    accum_out=dsum[:, 0:1],
                )
                absorb_dve(e)
                nc.vector.reciprocal(dinv[:, :], dsum[:, :])
                return e, dinv

            def vote(e, out=None):
                """acc = sum_k e[:,k] * P[:,k,:]: two interleaved DVE chains.
                If `out` is given, the combining add writes it (any dtype)."""
                acca = fresh([PP, D], F32, "vca")
                accb = fresh([PP, D], F32, "vcb")
                vs = out if out is not None else fresh([PP, D], F32, "vss")
                nc.vector.tensor_scalar(
                    out=acca[:, :], in0=Pv[:, 0, :], scalar1=e[:, 0:1],
                    scalar2=None, op0=ALU.mult,
                )
                nc.vector.tensor_scalar(
                    out=accb[:, :], in0=Pv[:, 1, :], scalar1=e[:, 1:2],
                    scalar2=None, op0=ALU.mult,
                )
                for k in range(2, A):
                    acc = acca if k % 2 == 0 else accb
                    nc.vector.scalar_tensor_tensor(
                        out=acc[:, :], in0=Pv[:, k, :], scalar=e[:, k : k + 1],
                        in1=acc[:, :], op0=ALU.mult, op1=ALU.add,
                    )
                nc.vector.tensor_tensor(
                    out=vs[:, :], in0=acca[:, :], in1=accb[:, :], op=ALU.add
                )
                return vs

            # ---- iteration 1: probs uniform over allowed; vote1 = vs1/A ----
            # agreement with raw v (squash factor f folded at the logit step:
            # <P_k, o> = f * <P_k, v>)
            f1 = squash_factor(vs1, inv_a2, inv_a)
            aT1 = fresh([PP, A], F32, "aT1")
            agreement(vs1h, aT1)
            logit1 = fresh([PP, A], F32, "lg1")
            nc.vector.tensor_scalar(
                out=logit1[:, :], in0=aT1[:, :], scalar1=f1[:, 0:1],
                scalar2=None, op0=ALU.mult,
            )  # tsv multiply is identity for allowed adapters

            # ---- iteration 2 ----
            e2, dinv2 = softmax(logit1)
            vs2 = vote(e2)
            vs2h = fresh([PP, D], FP16, "v2h")
            nc.vector.tensor_copy(vs2h[:, :], vs2[:, :])
            d2 = fresh([PP, 1], F32, "d2")
            nc.vector.tensor_tensor(
                out=d2[:, :], in0=dinv2[:, :], in1=dinv2[:, :], op=ALU.mult
            )
            f2 = squash_factor(vs2, d2[:, 0:1], dinv2[:, 0:1])
            aT2 = fresh([PP, A], F32, "aT2")
            agreement(vs2h, aT2)
            logit2 = fresh([PP, A], F32, "lg2")
            nc.vector.scalar_tensor_tensor(
                out=logit2[:, :], in0=aT2[:, :], scalar=f2[:, 0:1],
                in1=logit1[:, :], op0=ALU.mult, op1=ALU.add,
            )

            # ---- iteration 3: final vote = softmax-weighted mean, with the
            # 1/sum(e) normalization folded into the weights so the vote's
            # combining add writes bf16 u directly ----
            e3, dinv3 = softmax(logit2)
            e3n = fresh([PP, A], F32, "e3n")
            nc.vector.tensor_scalar(
                out=e3n[:, :], in0=e3[:, :], scalar1=dinv3[:, 0:1],
                scalar2=None, op0=ALU.mult,
            )
            v3h = fresh([PP, D], BF16, "v3h")
            vote(e3n, out=v3h)

            # ---- deinterleave the flat vote stream into u^T rows ----
            # vote [96,200] -> [32,600] (3 pairs per partition = 600 flat
            # values) -> stride-3 in-partition deinterleave -> [3, 6400].
            vstack = fresh([PP // CAPS, CAPS * D], BF16, "vstk")
            nc.gpsimd.dma_start(
                out=vstack[:, :].rearrange("q (m d) -> q m d", m=CAPS),
                in_=v3h[:, :],
            )
            uT2 = fresh([PP // CAPS, CAPS * D], BF16, "uT2")
            nc.vector.tensor_copy(
                uT2[:, :].rearrange("q (k jl) -> q k jl", k=CAPS),
                vstack[:, :].rearrange("q (jl k) -> q k jl", k=CAPS),
            )
            uT = sb.tile([CAPS, ROWS_PER_CORE], BF16, tag="uT")
            for kk, eng in enumerate((nc.gpsimd, nc.sync, nc.scalar)):
                eng.dma_start(
                    out=uT[kk : kk + 1, :].rearrange(
                        "k (q jl) -> k q jl", q=PP // CAPS
                    ),
                    in_=uT2[:, kk * D : (kk + 1) * D],
                )

            # PE absorbers: junk matmuls ladder the uT-writer + lwg ticks into
            # PE's clock (dep tracking is byte-range based)
            ps_junk = ps_pool.tile([1, 1], F32, tag="pjunk", bufs=1)
            for labs in (lwg_t[0:1, 0:1], uT[0:1, 0:1]):
                nc.tensor.matmul(ps_junk[:, :], labs, labs, start=True, stop=True)

            # ---- projection: out[j, :] = uT[:, j].T @ lwg ----
            # First batches are small so the store stream starts early.
            HM = M // 2
            BATCHES = [2, 3] + [5] * ((JCH - 5) // 5)
            assert sum(BATCHES) == JCH
            last_pab = None
            jc = 0
            for bt, bch in enumerate(BATCHES):
                if last_pab is not None:
                    absorb_dve(last_pab)
                    absorb_act(last_pab)
                osb = ob_pool.tile([128, 5 * M], F32, tag="osb", name="osb")
                for ji in range(bch):
                    js = jc * 128
                    co = ji * M
                    psA = ps_pool.tile([128, HM], F32, tag="psA", name="psA")
                    psB = ps_pool.tile([128, HM], F32, tag="psB", name="psB")
                    nc.tensor.matmul(
                        psA[:, :], uT[:, js : js + 128], lwg_t[:, :HM],
                        start=True, stop=True,
                    )
                    nc.tensor.matmul(
                        psB[:, :], uT[:, js : js + 128], lwg_t[:, HM:],
                        start=True, stop=True,
                    )
                    if ji == 0:
                        absorb_dve(psA)
                        absorb_act(psB)
                    nc.vector.tensor_copy(osb[:, co : co + HM], psA[:, :])
                    nc.scalar.copy(osb[:, co + HM : co + M], psB[:, :])
                    jc += 1
                r0 = (jc - bch) * 128
                src = osb[:, : bch * M].rearrange("p (j m) -> p j m", j=bch)
                dst = outc[r0 : r0 + bch * 128, :].rearrange("(j p) m -> p j m", p=128)
                pab = fresh([1, 2 * bch], F32, "pba")
                nc.gpsimd.tensor_copy(pab[:, :], osb[0:1, 0 : bch * M : HM])
                nc.gpsimd.dma_start(out=dst, in_=src)
                last_pab = pab
            _sb_cm.__exit__(None, None, None)
    return nc


def _get_programs(A, ka):
    key = (A, ka)
    if key not in _BUILD_CACHE:
        nc1, nc2 = _build_phase1(ka), _build_phase2(A)
        _split_multiwait_waits(nc1)
        _split_multiwait_waits(nc2)
        _BUILD_CACHE[key] = (nc1, nc2)
    return _BUILD_CACHE[key]


def kernel(t, x, s, route_weights, larger_w, larger_b, elarger, tsv):
    t = int(t)
    x = np.ascontiguousarray(np.asarray(x, np.float32))
    tsv_t = np.asarray(tsv, np.float32)[t]
    allowed = np.nonzero(tsv_t != 0)[0]
    assert np.all(tsv_t[allowed] == 1.0), "non-binary tsv not supported"
    A = len(allowed)
    ka = (A + NC - 1) // NC

    nc1, nc2 = _get_programs(A, ka)

    # ---------- phase 1: priors, expert-parallel ----------
    rw = np.asarray(route_weights, np.float32)
    in1 = []
    for c in range(NC):
        xw_c = np.zeros((ka, 128, NKC, XW_W), np.float16)
        for j in range(ka):
            g = c * ka + j
            if g < A:
                k = allowed[g]
                xT = x[:, k, :].T  # [600, 256]
                xh = xT.astype(np.float16)
                xl = (xT - xh.astype(np.float32)).astype(np.float16)
                W16 = rw[k].transpose(1, 0, 2).reshape(INCH, ND).astype(np.float16)
                for ci, (c0, cs) in enumerate(_K_CHUNKS):
                    xw_c[j, :cs, ci, :B] = xh[c0 : c0 + cs]
                    xw_c[j, :cs, ci, B : 2 * B] = xl[c0 : c0 + cs]
                    xw_c[j, :cs, ci, 2 * B :] = W16[c0 : c0 + cs]
        in1.append({"xw": xw_c.reshape(ka, 128, NKC * XW_W)})
    res1 = run_bass_kernel_spmd(nc1, in1, list(range(NC)))
    LAST_RESULTS.append(res1)

    # priors_full[k, b, n, d]; vote-1 numerator summed on device
    priors_full = np.zeros((A, B, CAPS, D), np.float32)
    vsum = np.zeros((2, 128, ND), np.float32)
    for c in range(NC):
        pri = res1.results[c]["pri"]  # [ka, 2, 128, 600]
        for j in range(ka):
            g = c * ka + j
            if g < A:
                priors_full[g] = pri[j].reshape(B, CAPS, D)
        vsum += res1.results[c]["S"]
    vsum_bnd = vsum.reshape(B, CAPS, D)

    # ---------- phase 2: routing + projection, pair-parallel ----------
    g_gate = 1.0 / (
        1.0 + np.exp(-(np.float32(s[0]) * np.asarray(elarger, np.float32)[t]))
    )
    lwg_f = np.asarray(larger_w, np.float32) * g_gate[:, None]  # [768, 3]
    bg = np.asarray(larger_b, np.float32) * g_gate  # [768]
    assert not np.any(bg), "nonzero larger_b not supported by this build"
    lwg_bf = lwg_f.T.astype(ml_dtypes.bfloat16)  # [3, 768]

    in2 = []
    for c in range(NC):
        sidx = np.arange(c * PP, (c + 1) * PP)
        nv, bv = sidx // B, sidx % B
        P2 = priors_full[:, bv, nv, :].transpose(1, 0, 2)  # [96, A, 200]
        in2.append(
            {
                "pri2": np.ascontiguousarray(
                    P2.reshape(PP, A * D).astype(np.float16)
                ),
                "vs1i": np.ascontiguousarray(vsum_bnd[bv, nv, :]),
                "lwg": lwg_bf,
            }
        )
    res2 = run_bass_kernel_spmd(nc2, in2, list(range(NC)))
    LAST_RESULTS.append(res2)

    out = np.concatenate([res2.results[c]["outc"] for c in range(NC)], axis=0)
    return out.reshape(B, D, M)


# revision 29
# speedup vs baseline: 1.1028x; 1.0451x over previous
"""Trainium2 Bass kernel for nn_CapsuleLayerTSV (capsule routing over 40 adapters).

Strategy (8 NeuronCores, two SPMD NEFFs, no collectives):
  Phase 1 (expert-parallel): allowed adapters (tsv[t] != 0) sharded across
    cores. Priors computed in fp16 with 2-term error compensation
    (xh@Wh + xl@Wh; W's fp16 rounding alone contributes ~1e-4 relative to
    priors, amplified ~30x by the routing softmax to ~9e-3 final -- inside
    the 2e-2 gate). fp16 matmuls run 1 cyc/row vs fp32's 4. Each core also
    emits S = sum of its adapters' priors on the otherwise-idle GpSimd so
    the iteration-1 uniform vote never has to be summed in phase 2.
  Host: reassemble priors, re-shard by the OUTPUT's flat row space (the torch
    .view(bz, 200, 3) scramble means output row r uses flat vote elements
    3r..3r+2, so core c needs pairs s in [96c, 96c+96), s = n*256 + b).
  Phase 2 (pair-parallel): 3-iteration dynamic routing with the squash factor
    folded into the agreement ops as a per-partition scalar (o_i is never
    materialized), 21-term sweeps split Vector(14)/GpSimd(7), softmax's
    subtract fused into the ACT exp via bias=-rowmax, then a bf16 projection
    u[6400,3] @ (larger_w*g).T streamed out in 5-j-chunk batches at HBM line
    rate.
"""

import sys

sys.path.insert(0, "/opt/trn_rl_repo")

import numpy as np
import ml_dtypes

import concourse.bass as bass
import concourse.mybir as mybir
import concourse.tile as tile
from concourse.bass_utils import run_bass_kernel_spmd

F32 = mybir.dt.float32
BF16 = mybir.dt.bfloat16
FP16 = mybir.dt.float16
AX = mybir.AxisListType
ALU = mybir.AluOpType
ACTF = mybir.ActivationFunctionType

NC = 8
B = 256
ADP = 40
CAPS = 3
INCH = 600
D = 200
M = 768
ND = CAPS * D  # 600
PP = CAPS * B // NC  # 96 (n,b) pairs per core in phase 2
ROWS_PER_CORE = PP * D // CAPS  # 6400 output rows per core
JCH = ROWS_PER_CORE // 128  # 50 j-chunks

_K_CHUNKS = [(0, 128), (128, 128), (256, 128), (384, 128), (512, 88)]
NKC = len(_K_CHUNKS)
XW_W = 2 * B + ND  # 1112: [xh^T | xl^T | Wh] per k-chunk

_BUILD_CACHE = {}


def _split_multiwait_waits(nc):
    """walrus caps sync-waits at ONE per instruction. For instructions executed
    by an in-order engine sequencer (everything except queue-executed DMAs),
    splitting the wait list across preceding 1-wait NoOps/Drains on the same
    engine is semantics-preserving."""
    for fn in nc.m.functions:
        for blk in fn.blocks:
            out = []
            for inst in blk.instructions:
                si = getattr(inst, "sync_info", None)
                if (
                    si is not None
                    and si.on_wait
                    and len(si.on_wait) > 1
                    and not isinstance(inst, mybir.InstDMACopy)
                    and getattr(inst, "engine", None) is not None
                ):
                    waits = list(si.on_wait)
                    cls = (
                        mybir.InstDrain
                        if isinstance(inst, mybir.InstDrain)
                        else mybir.InstNoOp
                    )
                    for i, w in enumerate(waits[:-1]):
                        extra = cls(
                            name=f"{inst.name}_w{i}",
                            engine=inst.engine,
                            sync_info=mybir.SyncInfo(on_wait=[w], on_update=[]),
                            bass_nofuse=True,
                        )
                        nc.register_instruction(extra)
                        out.append(extra)
                    si.on_wait = waits[-1:]
                out.append(inst)
            blk.instructions = out


# test/debug hook: kernel() appends the BassKernelResults of each phase here
LAST_RESULTS = []


def _build_phase1(ka):
    """SPMD program: fp16 2-term priors for `ka` adapter slots per core.

    inputs : xw  [ka, 128, 5*1112] fp16 -- per k-chunk ci the column block
             [ci*1112, (ci+1)*1112) holds [xh^T | xl^T | Wh], rows past the
             chunk's K zero-padded (zero rows accumulate nothing)
    outputs: pri [ka, 2, 128, 600] f32  (priors, b in 2 chunks of 128)
             S   [2, 128, 600] f32      (sum of this core's adapters' priors)
    """
    nc = bass.Bass()
    xw = nc.declare_dram_parameter("xw", [ka, 128, NKC * XW_W], FP16, isOutput=False)
    pri = nc.declare_dram_parameter("pri", [ka, 2, 128, ND], F32, isOutput=True)
    S = nc.declare_dram_parameter("S", [2, 128, ND], F32, isOutput=True)

    with tile.TileContext(nc) as tc:
        with (
            tc.tile_pool(name="xt", bufs=1) as xt_pool,
            tc.tile_pool(name="ob", bufs=2 * ka) as ob_pool,
            tc.tile_pool(name="ssb", bufs=1) as s_pool,
            tc.tile_pool(name="ps", bufs=2, space="PSUM") as ps_pool,
        ):
            # Wait-budget discipline (walrus: max ONE sync-wait per
            # instruction): junk [1,1] matmuls absorb each DMA tick into
            # PE's observed clock, so real matmuls only ever wait on the PSUM
            # slot release. Output tiles are never reused (bufs=2*ka).
            ps_junk = ps_pool.tile([1, 1], F32, tag="pjunk", bufs=1)
            S_sb = [
                s_pool.tile([128, ND], F32, tag=f"S{bc}", name=f"S{bc}")
                for bc in range(2)
            ]
            for k in range(ka):
                xw_t = xt_pool.tile(
                    [128, NKC * XW_W], FP16, tag=f"xw{k}", name=f"xw{k}"
                )
                if k == 0:
                    # adapter 0 loads in per-chunk slices: queued DMAs share
                    # the ring round-robin, so one big DMA would delay the
                    # first matmul by the FULL transfer; slices release the
                    # first chunk early. Junk matmuls absorb each DMA tick
                    # and keep the PE HAM warm during the wait.
                    for ci in range(NKC):
                        nc.sync.dma_start(
                            out=xw_t[:, ci * XW_W : (ci + 1) * XW_W],
                            in_=xw[k, :, ci * XW_W : (ci + 1) * XW_W],
                        )
                        nc.tensor.matmul(
                            ps_junk[:, :],
                            xw_t[0:1, ci * XW_W : ci * XW_W + 1],
                            xw_t[0:1, ci * XW_W : ci * XW_W + 1],
                            start=True, stop=True,
                        )
                else:
                    nc.sync.dma_start(out=xw_t[:, :], in_=xw[k, :, :])
                    nc.tensor.matmul(
                        ps_junk[:, :], xw_t[0:1, 0:1], xw_t[0:1, 0:1],
                        start=True, stop=True,
                    )
                for bc in range(2):
                    pss = [
                        ps_pool.tile([128, ND // 2], F32, tag=f"ps{gi}", name=f"ps{gi}")
                        for gi in range(2)
                    ]
                    for ci in range(NKC):
                        c0 = ci * XW_W
                        for ti in range(2):  # hi, lo term of x
                            xoff = c0 + ti * B + bc * 128
                            for gi in range(2):
                                nc.tensor.matmul(
                                    pss[gi][:, :],
                                    xw_t[:, xoff : xoff + 128],
                                    xw_t[
                                        :,
                                        c0 + 2 * B + gi * 300 : c0
                                        + 2 * B
                                        + (gi + 1) * 300,
                                    ],
                                    start=(ci == 0 and ti == 0),
                                    stop=(ci == NKC - 1 and ti == 1),
                                )
                    osb = ob_pool.tile([128, ND], F32, tag="osb")
                    nc.vector.tensor_copy(osb[:, :300], pss[0][:, :])
                    nc.vector.tensor_copy(osb[:, 300:], pss[1][:, :])
                    # iter-1 vote partial sum (DVE: GpSimd's 2-input ops are
                    # ~3x slower and trailed the matmuls by ~12us)
                    if k == 0:
                        nc.vector.tensor_copy(S_sb[bc][:, :], osb[:, :])
                    else:
                        nc.vector.tensor_tensor(
                            out=S_sb[bc][:, :], in0=S_sb[bc][:, :], in1=osb[:, :],
                            op=ALU.add,
                        )
                    # GP absorber pulls the DVE tick so the store has <=1 wait
                    pabs = ob_pool.tile(
                        [1, 1], F32, tag=f"pa{k}_{bc}", name=f"pa{k}_{bc}", bufs=1
                    )
                    nc.gpsimd.tensor_copy(pabs[:, :], osb[0:1, ND - 1 : ND])
                    nc.gpsimd.dma_start(out=pri[k, bc, :, :], in_=osb[:, :])
            for bc in range(2):
                sabs = ob_pool.tile(
                    [1, 1], F32, tag=f"sa{bc}", name=f"sa{bc}", bufs=1
                )
                nc.gpsimd.tensor_copy(sabs[:, :], S_sb[bc][0:1, ND - 1 : ND])
                nc.gpsimd.dma_start(out=S[bc, :, :], in_=S_sb[bc][:, :])
    return nc


def _build_phase2(A):
    """SPMD program: routing for 96 (n,b) pairs + output projection per core.

    inputs : pri2 [96, A*200] f32  (priors for this core's pairs, k-major)
             vs1i [96, 200] f32    (sum over allowed k of priors = A*vote_1)
             lwg  [3, 768] bf16    (gated projection matrix, plain bf16)
    output : outc [6400, 768] f32

    Assumes tsv[t, allowed] == 1 (checked on host), so the reference's
    logits*tsv multiplies are identity for every adapter we process.
    """
    nc = bass.Bass()
    pri2 = nc.declare_dram_parameter("pri2", [PP, A * D], FP16, isOutput=False)
    vs1i = nc.declare_dram_parameter("vs1i", [PP, D], F32, isOutput=False)
    lwg = nc.declare_dram_parameter("lwg", [CAPS, M], BF16, isOutput=False)
    outc = nc.declare_dram_parameter("outc", [ROWS_PER_CORE, M], F32, isOutput=True)

    inv_a = 1.0 / float(A)
    inv_a2 = inv_a * inv_a
    uid = [0]
    DVK = 14  # adapters 0:DVK on Vector, DVK:A on GpSimd per sweep

    with tile.TileContext(nc) as tc:
        with (
            tc.tile_pool(name="ps", bufs=2, space="PSUM") as ps_pool,
            tc.tile_pool(name="ob", bufs=3) as ob_pool,
        ):
            _sb_cm = tc.tile_pool(name="sb", bufs=1)
            sb = _sb_cm.__enter__()

            def fresh(shape, dtype=F32, pfx="t", pool=None):
                uid[0] += 1
                p = pool if pool is not None else sb
                return p.tile(shape, dtype, tag=f"{pfx}{uid[0]}", name=f"{pfx}{uid[0]}")

            def absorb_dve(ap):
                s = fresh([1, 1], ap.dtype, "slv")
                nc.vector.tensor_copy(s[:, :], ap[0:1, 0:1])

            def absorb_act(ap):
                s = fresh([1, 1], ap.dtype, "sla")
                nc.scalar.copy(s[:, :], ap[0:1, 0:1])

            def absorb_gp(ap):
                s = fresh([1, 1], ap.dtype, "slg")
                nc.gpsimd.tensor_copy(s[:, :], ap[0:1, 0:1])

            # ---- ACT table warmup: exp+sqrt tables load during the P DMA ----
            warm = fresh([1, 1], F32, "wrm")
            warm2 = fresh([1, 1], F32, "wr2")
            nc.vector.memset(warm[:, :], 1.0)
            absorb_act(warm)
            nc.scalar.activation(warm2[:, :], warm[:, :], ACTF.Exp)
            nc.scalar.sqrt(warm2[:, :], warm[:, :])

            # ---- loads: vs1 first (tiny, gates f1), then P across both
            # HWDGE rings in 4 slices ----
            vs1 = sb.tile([PP, D], F32, tag="vs1")
            nc.sync.dma_start(out=vs1[:, :], in_=vs1i[:, :])
            P = sb.tile([PP, A * D], FP16, tag="P")
            Pv = P[:, :].rearrange("p (k d) -> p k d", k=A)
            QK = [(0, 5), (5, 10), (10, 16), (16, A)]
            for qi, (k0, k1) in enumerate(QK):
                eng = nc.sync if qi % 2 == 0 else nc.scalar
                eng.dma_start(
                    out=P[:, k0 * D : k1 * D], in_=pri2[:, k0 * D : k1 * D]
                )
            absorb_dve(vs1)
            lwg_t = sb.tile([CAPS, M], BF16, tag="lwg")
            nc.scalar.dma_start(out=lwg_t[:, :], in_=lwg[:, :])
            for _ in range(3):
                absorb_dve(P)  # pull the four P-DMA ticks into DVE's clock
            pabP = fresh([1, 1], FP16, "pgp")
            nc.gpsimd.tensor_copy(pabP[:, :], P[0:1, A * D - 1 : A * D])
            vs1h = fresh([PP, D], FP16, "v1h")
            nc.vector.tensor_copy(vs1h[:, :], vs1[:, :])

            def squash_factor(v_t, pre_scale_sq, post_scale):
                """f = post_scale * sqrt(t)/(1+t), t = sum(v_t^2)*pre_scale_sq.
                pre_scale_sq/post_scale: float or [PP,1] AP."""
                junk = fresh([PP, D], F32, "sqj")
                sq = fresh([PP, 1], F32, "sq")
                t = fresh([PP, 1], F32, "tt")
                s = fresh([PP, 1], F32, "ss")
                u = fresh([PP, 1], F32, "uu")
                r = fresh([PP, 1], F32, "rr")
                f = fresh([PP, 1], F32, "ff")
                nc.vector.scalar_tensor_tensor(
                    out=junk[:, :], in0=v_t[:, :], scalar=1.0, in1=v_t[:, :],
                    op0=ALU.mult, op1=ALU.mult, accum_out=sq[:, 0:1],
                )
                if isinstance(pre_scale_sq, float):
                    nc.vector.tensor_scalar(
                        out=t[:, :], in0=sq[:, :], scalar1=pre_scale_sq,
                        scalar2=None, op0=ALU.mult,
                    )
                else:
                    nc.vector.tensor_tensor(
                        out=t[:, :], in0=sq[:, :], in1=pre_scale_sq, op=ALU.mult
                    )
                nc.scalar.sqrt(s[:, :], t[:, :])
                nc.vector.tensor_scalar(
                    out=u[:, :], in0=t[:, :], scalar1=1.0, scalar2=None, op0=ALU.add,
                )
                nc.vector.reciprocal(r[:, :], u[:, :])
                absorb_dve(s)  # pull the ACT sqrt tick before the fused f op
                nc.vector.scalar_tensor_tensor(
                    out=f[:, :], in0=s[:, :], scalar=post_scale, in1=r[:, :],
                    op0=ALU.mult, op1=ALU.mult,
                )
                return f

            def agreement(v16, aT):
                """aT[:, k] = raw sum_d P[:,k,:] * v16: ONE broadcast multiply
                (fp16 everywhere -> DVE 2x mode) + ONE segmented X-reduce per
                half-k slice (halves bound the op latency)."""
                AH = A // 2
                for k0, k1 in ((0, AH), (AH, A)):
                    kk = k1 - k0
                    prod = fresh([PP, kk * D], FP16, "agp")
                    pv3 = prod[:, :].rearrange("p (k d) -> p k d", k=kk)
                    nc.vector.tensor_tensor(
                        out=pv3,
                        in0=Pv[:, k0:k1, :],
                        in1=v16[:, :].unsqueeze(1).broadcast_to([PP, kk, D]),
                        op=ALU.mult,
                    )
                    nc.vector.tensor_reduce(aT[:, k0:k1], pv3, AX.X, ALU.add)

            def softmax(logit):
                """returns (e, dinv): e = exp(logit - max), dinv = 1/sum(e)."""
                rmax = fresh([PP, 1], F32, "rmx")
                nmx = fresh([PP, 1], F32, "nmx")
                e = fresh([PP, A], F32, "e")
                dsum = fresh([PP, 1], F32, "dsm")
                dinv = fresh([PP, 1], F32, "dnv")
                nc.vector.tensor_reduce(rmax[:, :], logit[:, :], AX.X, ALU.max)
                nc.vector.tensor_scalar(
                    out=nmx[:, :], in0=rmax[:, :], scalar1=-1.0, scalar2=None,
                    op0=ALU.mult,
                )
                absorb_act(nmx)  # ACT waits once on DVE tick, then exp is clean
                nc.scalar.activation(
                    e[:, :], logit[:, :], ACTF.Exp, bias=nmx[:, 0:1],
                    accum_out=dsum[:, 0:1],
                )
                absorb_dve(e)
                nc.vector.reciprocal(dinv[:, :], dsum[:, :])
                return e, dinv

            def vote(e, out=None):
                """acc = sum_k e[:,k] * P[:,k,:]: two interleaved DVE chains.
                If `out` is given, the combining add writes it (any dtype)."""
                acca = fresh([PP, D], F32, "vca")
                accb = fresh([PP, D], F32, "vcb")
                vs = out if out is not None else fresh([PP, D], F32, "vss")
                nc.vector.tensor_scalar(
                    out=acca[:, :], in0=Pv[:, 0, :], scalar1=e[:, 0:1],
                    scalar2=None, op0=ALU.mult,
                )
                nc.vector.tensor_scalar(
                    out=accb[:, :], in0=Pv[:, 1, :], scalar1=e[:, 1:2],
                    scalar2=None, op0=ALU.mult,
                )
                for k in range(2, A):
                    acc = acca if k % 2 == 0 else accb
                    nc.vector.scalar_tensor_tensor(
                        out=acc[:, :], in0=Pv[:, k, :], scalar=e[:, k : k + 1],
                        in1=acc[:, :], op0=ALU.mult, op1=ALU.add,
                    )
                nc.vector.tensor_tensor(
                    out=vs[:, :], in0=acca[:, :], in1=accb[:, :], op=ALU.add
                )
                return vs

            # ---- iteration 1: probs uniform over allowed; vote1 = vs1/A ----
            # agreement with raw v (squash factor f folded at the logit step:
            # <P_k, o> = f * <P_k, v>)
            f1 = squash_factor(vs1, inv_a2, inv_a)
            aT1 = fresh([PP, A], F32, "aT1")
            agreement(vs1h, aT1)
            logit1 = fresh([PP, A], F32, "lg1")
            nc.vector.tensor_scalar(
                out=logit1[:, :], in0=aT1[:, :], scalar1=f1[:, 0:1],
                scalar2=None, op0=ALU.mult,
            )  # tsv multiply is identity for allowed adapters

            # ---- iteration 2 ----
            e2, dinv2 = softmax(logit1)
            vs2 = vote(e2)
            vs2h = fresh([PP, D], FP16, "v2h")
            nc.vector.tensor_copy(vs2h[:, :], vs2[:, :])
            d2 = fresh([PP, 1], F32, "d2")
            nc.vector.tensor_tensor(
                out=d2[:, :], in0=dinv2[:, :], in1=dinv2[:, :], op=ALU.mult
            )
            f2 = squash_factor(vs2, d2[:, 0:1], dinv2[:, 0:1])
            aT2 = fresh([PP, A], F32, "aT2")
            agreement(vs2h, aT2)
            logit2 = fresh([PP, A], F32, "lg2")
            nc.vector.scalar_tensor_tensor(
                out=logit2[:, :], in0=aT2[:, :], scalar=f2[:, 0:1],
                in1=logit1[:, :], op0=ALU.mult, op1=ALU.add,
            )

            # ---- iteration 3: final vote = softmax-weighted mean, with the
            # 1/sum(e) normalization folded into the weights so the vote's
            # combining add writes bf16 u directly ----
            e3, dinv3 = softmax(logit2)
            e3n = fresh([PP, A], F32, "e3n")
            nc.vector.tensor_scalar(
                out=e3n[:, :], in0=e3[:, :], scalar1=dinv3[:, 0:1],
                scalar2=None, op0=ALU.mult,
            )
            v3h = fresh([PP, D], BF16, "v3h")
            vote(e3n, out=v3h)

            # ---- deinterleave the flat vote stream into u^T rows ----
            # vote [96,200] -> [32,600] (3 pairs per partition = 600 flat
            # values) -> stride-3 in-partition deinterleave -> [3, 6400].
            vstack = fresh([PP // CAPS, CAPS * D], BF16, "vstk")
            nc.gpsimd.dma_start(
                out=vstack[:, :].rearrange("q (m d) -> q m d", m=CAPS),
                in_=v3h[:, :],
            )
            uT2 = fresh([PP // CAPS, CAPS * D], BF16, "uT2")
            nc.vector.tensor_copy(
                uT2[:, :].rearrange("q (k jl) -> q k jl", k=CAPS),
                vstack[:, :].rearrange("q (jl k) -> q k jl", k=CAPS),
            )
            uT = sb.tile([CAPS, ROWS_PER_CORE], BF16, tag="uT")
            for kk, eng in enumerate((nc.gpsimd, nc.sync, nc.scalar)):
                eng.dma_start(
                    out=uT[kk : kk + 1, :].rearrange(
                        "k (q jl) -> k q jl", q=PP // CAPS
                    ),
                    in_=uT2[:, kk * D : (kk + 1) * D],
                )

            # PE absorbers: junk matmuls ladder the uT-writer + lwg ticks into
            # PE's clock (dep tracking is byte-range based)
            ps_junk = ps_pool.tile([1, 1], F32, tag="pjunk", bufs=1)
            for labs in (lwg_t[0:1, 0:1], uT[0:1, 0:1]):
                nc.tensor.matmul(ps_junk[:, :], labs, labs, start=True, stop=True)

            # ---- projection: out[j, :] = uT[:, j].T @ lwg ----
            # First batches are small so the store stream starts early.
            HM = M // 2
            BATCHES = [1, 2, 2] + [5] * ((JCH - 5) // 5)
            assert sum(BATCHES) == JCH
            last_pab = None
            jc = 0
            for bt, bch in enumerate(BATCHES):
                if last_pab is not None:
                    absorb_dve(last_pab)
                    absorb_act(last_pab)
                osb = ob_pool.tile([128, 5 * M], F32, tag="osb", name="osb")
                for ji in range(bch):
                    js = jc * 128
                    co = ji * M
                    psA = ps_pool.tile([128, HM], F32, tag="psA", name="psA")
                    psB = ps_pool.tile([128, HM], F32, tag="psB", name="psB")
                    nc.tensor.matmul(
                        psA[:, :], uT[:, js : js + 128], lwg_t[:, :HM],
                        start=True, stop=True,
                    )
                    nc.tensor.matmul(
                        psB[:, :], uT[:, js : js + 128], lwg_t[:, HM:],
                        start=True, stop=True,
                    )
                    if ji == 0:
                        absorb_dve(psA)
                        absorb_act(psB)
                    nc.vector.tensor_copy(osb[:, co : co + HM], psA[:, :])
                    nc.scalar.copy(osb[:, co + HM : co + M], psB[:, :])
                    jc += 1
                r0 = (jc - bch) * 128
                src = osb[:, : bch * M].rearrange("p (j m) -> p j m", j=bch)
                dst = outc[r0 : r0 + bch * 128, :].rearrange("(j p) m -> p j m", p=128)
                pab = fresh([1, 2 * bch], F32, "pba")
                nc.gpsimd.tensor_copy(pab[:, :], osb[0:1, 0 : bch * M : HM])
                nc.gpsimd.dma_start(out=dst, in_=src)
                last_pab = pab
            _sb_cm.__exit__(None, None, None)
    return nc


def _get_programs(A, ka):
    key = (A, ka)
    if key not in _BUILD_CACHE:
        nc1, nc2 = _build_phase1(ka), _build_phase2(A)
        _split_multiwait_waits(nc1)
        _split_multiwait_waits(nc2)
        _BUILD_CACHE[key] = (nc1, nc2)
    return _BUILD_CACHE[key]


def kernel(t, x, s, route_weights, larger_w, larger_b, elarger, tsv):
    t = int(t)
    x = np.ascontiguousarray(np.asarray(x, np.float32))
    tsv_t = np.asarray(tsv, np.float32)[t]
    allowed = np.nonzero(tsv_t != 0)[0]
    assert np.all(tsv_t[allowed] == 1.0), "non-binary tsv not supported"
    A = len(allowed)
    ka = (A + NC - 1) // NC

    nc1, nc2 = _get_programs(A, ka)

    # ---------- phase 1: priors, expert-parallel ----------
    rw = np.asarray(route_weights, np.float32)
    in1 = []
    for c in range(NC):
        xw_c = np.zeros((ka, 128, NKC, XW_W), np.float16)
        for j in range(ka):
            g = c * ka + j
            if g < A:
                k = allowed[g]
                xT = x[:, k, :].T  # [600, 256]
                xh = xT.astype(np.float16)
                xl = (xT - xh.astype(np.float32)).astype(np.float16)
                W16 = rw[k].transpose(1, 0, 2).reshape(INCH, ND).astype(np.float16)
                for ci, (c0, cs) in enumerate(_K_CHUNKS):
                    xw_c[j, :cs, ci, :B] = xh[c0 : c0 + cs]
                    xw_c[j, :cs, ci, B : 2 * B] = xl[c0 : c0 + cs]
                    xw_c[j, :cs, ci, 2 * B :] = W16[c0 : c0 + cs]
        in1.append({"xw": xw_c.reshape(ka, 128, NKC * XW_W)})
    res1 = run_bass_kernel_spmd(nc1, in1, list(range(NC)))
    LAST_RESULTS.append(res1)

    # priors_full[k, b, n, d]; vote-1 numerator summed on device
    priors_full = np.zeros((A, B, CAPS, D), np.float32)
    vsum = np.zeros((2, 128, ND), np.float32)
    for c in range(NC):
        pri = res1.results[c]["pri"]  # [ka, 2, 128, 600]
        for j in range(ka):
            g = c * ka + j
            if g < A:
                priors_full[g] = pri[j].reshape(B, CAPS, D)
        vsum += res1.results[c]["S"]
    vsum_bnd = vsum.reshape(B, CAPS, D)

    # ---------- phase 2: routing + projection, pair-parallel ----------
    g_gate = 1.0 / (
        1.0 + np.exp(-(np.float32(s[0]) * np.asarray(elarger, np.float32)[t]))
    )
    lwg_f = np.asarray(larger_w, np.float32) * g_gate[:, None]  # [768, 3]
    bg = np.asarray(larger_b, np.float32) * g_gate  # [768]
    assert not np.any(bg), "nonzero larger_b not supported by this build"
    lwg_bf = lwg_f.T.astype(ml_dtypes.bfloat16)  # [3, 768]

    in2 = []
    for c in range(NC):
        sidx = np.arange(c * PP, (c + 1) * PP)
        nv, bv = sidx // B, sidx % B
        P2 = priors_full[:, bv, nv, :].transpose(1, 0, 2)  # [96, A, 200]
        in2.append(
            {
                "pri2": np.ascontiguousarray(
                    P2.reshape(PP, A * D).astype(np.float16)
                ),
                "vs1i": np.ascontiguousarray(vsum_bnd[bv, nv, :]),
                "lwg": lwg_bf,
            }
        )
    res2 = run_bass_kernel_spmd(nc2, in2, list(range(NC)))
    LAST_RESULTS.append(res2)

    out = np.concatenate([res2.results[c]["outc"] for c in range(NC)], axis=0)
    return out.reshape(B, D, M)
